# revision 2
# baseline (speedup 1.0000x reference)
"""MoE AdaptiveExpertLayer on 8 TRN2 NeuronCores (expert-parallel Bass kernel).

Sharding (hardcoded): expert-parallel — core e owns expert e's W1/b1/W2/b2.
The router (gate matmul + softmax + top-2, ~0.01% of total FLOPs) runs on the
host during input sharding; tokens are dispatched to their two chosen experts'
cores as capacity-padded batches ("all-to-all dispatch by router choice" done
at the sharding step).  Each core runs the expert MLP
    y = (relu(x @ W1.T + b1) @ W2.T + b2) * combine_weight
over its C dispatched tokens, in bf16 with fp32 PSUM accumulation, weights
fully SBUF-resident.  The host sums each token's two expert contributions.

Problem shapes: x [4, 2048, 1024], W1 [8, 4096, 1024], W2 [8, 1024, 4096].

The PE matmul stream (2368 matmuls) runs back-to-back at the hardware's
sustained clock; the remaining levers are startup latency (first weights/x
tiles), the drain tail, and DMA instruction count.  All bulk loads are single
multi-k-chunk DMAs (3D access patterns) so each transfer is >=256KB and the
queues carry ~60 instructions instead of ~275.
"""

import time

import numpy as np
import ml_dtypes
from contextlib import ExitStack

import concourse.tile as tile
from concourse import bacc, mybir
from concourse.tile import add_dep_helper
from concourse.bass_utils import run_bass_kernel_spmd

D_MODEL = 1024
D_FF = 4096
N_EXPERTS = 8
TOP_K = 2
N_CORES = 8
CAPACITY = 2176  # default per-expert token capacity (multiple of 128)

BF16 = mybir.dt.bfloat16
F32 = mybir.dt.float32
_BF = ml_dtypes.bfloat16

# Set by callers that want NTFF profiling; BASS_TRACE=1 env also works.
TRACE = False
LAST_RESULTS = None

_graph_cache = {}


def _token_blocks(c):
    """Split capacity into matmul token-blocks of <=512 (multiples of 128)."""
    blocks = []
    t0 = 0
    while t0 < c:
        tb = min(512, c - t0)
        blocks.append((t0, tb))
        t0 += tb
    return blocks


def _build_graph(c):
    """Build + compile the per-core expert-MLP Bass graph for capacity c."""
    nc = bacc.Bacc("TRN2", target_bir_lowering=False, debug=False,
                   num_devices=N_CORES)

    xt = nc.dram_tensor("xt", [D_MODEL, c], BF16, kind="ExternalInput").ap()
    w1t = nc.dram_tensor("w1t", [D_MODEL, D_FF], BF16, kind="ExternalInput").ap()
    w2t = nc.dram_tensor("w2t", [D_FF, D_MODEL], BF16, kind="ExternalInput").ap()
    b1 = nc.dram_tensor("b1", [128, D_FF // 128], F32, kind="ExternalInput").ap()
    b2bc = nc.dram_tensor("b2bc", [128, D_MODEL], F32, kind="ExternalInput").ap()
    s = nc.dram_tensor("s", [128, c // 128], F32, kind="ExternalInput").ap()
    out = nc.dram_tensor("out", [c, D_MODEL], F32, kind="ExternalOutput").ap()

    n_k1 = D_MODEL // 128   # 8  contraction chunks for matmul 1
    n_m1 = D_FF // 128      # 32 output tiles for matmul 1
    n_dn = D_MODEL // 512   # 2  output column tiles for matmul 2

    # multi-k-chunk DRAM views: one DMA covers all contraction chunks
    xt3 = xt.rearrange("(k p) t -> p k t", k=n_k1)      # [128, 8, c]
    w1t3 = w1t.rearrange("(k p) f -> p k f", k=n_k1)    # [128, 8, 4096]
    w2t3 = w2t.rearrange("(k p) f -> p k f", k=n_m1)    # [128, 32, 1024]

    with tile.TileContext(nc) as tc, ExitStack() as ctx:
        wp1 = ctx.enter_context(tc.tile_pool(name="w1", bufs=1))
        wp2 = ctx.enter_context(tc.tile_pool(name="w2", bufs=1))
        cpool = ctx.enter_context(tc.tile_pool(name="consts", bufs=2))
        b2pool = ctx.enter_context(tc.tile_pool(name="b2p", bufs=1))
        xpool = ctx.enter_context(tc.tile_pool(name="x", bufs=2))
        hpool = ctx.enter_context(tc.tile_pool(name="h", bufs=n_m1))
        opool = ctx.enter_context(tc.tile_pool(name="o", bufs=4))
        pp1 = ctx.enter_context(tc.tile_pool(name="p1", bufs=5, space="PSUM"))
        pp2 = ctx.enter_context(tc.tile_pool(name="p2", bufs=3, space="PSUM"))

        blocks = _token_blocks(c)

        # w1 lives as one [128, 8*4096] tile; column segment j of every
        # k-chunk loads in a single 3D DMA.  The first segment is narrow so
        # the PE's first psum tile can start ~3us in; later segments are
        # 512 cols (1MB) and stay well ahead of the PE's 2.1us/128-col pace.
        w1b = wp1.tile([128, n_k1 * D_FF], BF16, tag="w1", name="w1b")
        w1bv = w1b[:].rearrange("p (k f) -> p k f", k=n_k1)
        seg_bounds = [0, 128, 512, 1024, 1536, 2048, 2560, 3072, 3584, D_FF]
        for lo, hi in zip(seg_bounds[:-1], seg_bounds[1:]):
            nc.sync.dma_start(w1bv[:, :, lo:hi], w1t3[:, :, lo:hi])

        b1_all = cpool.tile([128, n_m1], F32, tag="b1a", name="b1a")
        nc.gpsimd.dma_start(b1_all[:], b1[:, :])
        b1_tiles = [b1_all[:, m:m + 1] for m in range(n_m1)]

        # x block 0 on the scalar queue so it lands in parallel with w1 seg 0
        t0_0, tb_0 = blocks[0]
        x0 = xpool.tile([128, n_k1 * tb_0], BF16, tag="x", name="x0")
        x0v = x0[:].rearrange("p (k t) -> p k t", k=n_k1)
        nc.scalar.dma_start(x0v[:, :, :], xt3[:, :, t0_0:t0_0 + tb_0])

        w2b = wp2.tile([128, n_m1 * D_MODEL], BF16, tag="w2", name="w2b")
        w2bv = w2b[:].rearrange("p (k f) -> p k f", k=n_m1)
        s_all = cpool.tile([128, c // 128], F32, tag="sa", name="sa")
        b2_tile = b2pool.tile([128, D_MODEL], F32, tag="b2", name="b2t")

        x_tiles = x0
        x_next = None
        first = True
        for bi, (t0, tb) in enumerate(blocks):
            if not first:
                x_tiles = x_next

            # prefetch next block's x while this block computes
            if bi + 1 < len(blocks):
                tn0, tnb = blocks[bi + 1]
                x_next = xpool.tile([128, n_k1 * tnb], BF16, tag="x",
                                    name=f"x{bi + 1}")
                xnv = x_next[:].rearrange("p (k t) -> p k t", k=n_k1)
                nc.sync.dma_start(xnv[:, :, :], xt3[:, :, tn0:tn0 + tnb])

            # h.T [D_FF, tb] = relu(W1 @ x.T + b1), FF on partitions
            h_tiles = []
            relu_insts = []
            for m in range(n_m1):
                ps = pp1.tile([128, tb], F32, tag="p1", name=f"p1_{bi}_{m}")
                for k in range(n_k1):
                    nc.tensor.matmul(
                        ps[:],
                        lhsT=w1b[:, k * D_FF + m * 128:k * D_FF + (m + 1) * 128],
                        rhs=x_tiles[:, k * tb:(k + 1) * tb],
                        start=(k == 0), stop=(k == n_k1 - 1))
                h = hpool.tile([128, tb], BF16, tag="h", name=f"h_{bi}_{m}")
                ri = nc.scalar.activation(h[:], ps[:],
                                          mybir.ActivationFunctionType.Relu,
                                          bias=b1_tiles[m][:])
                relu_insts.append(ri)
                h_tiles.append(h)

            if first:
                # w2 / s / b2 only gate matmul 2 — load them behind early m1
                # on the gpsimd queue so the w1 segment loads (which the PE
                # is waiting on) get the HBM bandwidth first.
                first = False
                for kc in range(4):
                    d = nc.gpsimd.dma_start(w2bv[:, kc * 8:(kc + 1) * 8, :],
                                            w2t3[:, kc * 8:(kc + 1) * 8, :])
                    add_dep_helper(d.ins, relu_insts[4].ins, sync=True,
                                   reason="w2 load behind early m1")
                nc.gpsimd.dma_start(s_all[:], s[:, :])
                nc.gpsimd.dma_start(b2_tile[:], b2bc[:, :])

            # y [tb, D_MODEL] = (h @ W2.T + b2) * s, tokens on partitions
            for tm in range(tb // 128):
                g = (t0 + tm * 128) // 128
                for dn in range(n_dn):
                    ps = pp2.tile([128, 512], F32, tag="p2",
                                  name=f"p2_{bi}_{tm}_{dn}")
                    for k in range(n_m1):
                        nc.tensor.matmul(
                            ps[:], lhsT=h_tiles[k][:, tm * 128:(tm + 1) * 128],
                            rhs=w2b[:, k * D_MODEL + dn * 512:
                                    k * D_MODEL + (dn + 1) * 512],
                            start=(k == 0), stop=(k == n_m1 - 1))
                    t = opool.tile([128, 512], F32, tag="t",
                                   name=f"t_{bi}_{tm}_{dn}")
                    nc.vector.tensor_add(t[:], ps[:],
                                         b2_tile[:, dn * 512:(dn + 1) * 512])
                    o = opool.tile([128, 512], F32, tag="o",
                                   name=f"o_{bi}_{tm}_{dn}")
                    nc.scalar.mul(o[:], t[:], s_all[:, g:g + 1])
                    nc.sync.dma_start(
                        out[t0 + tm * 128:t0 + (tm + 1) * 128,
                            dn * 512:(dn + 1) * 512],
                        o[:])

    nc.compile()
    return nc


def _get_graph(c):
    if c not in _graph_cache:
        _graph_cache[c] = _build_graph(c)
    return _graph_cache[c]


def kernel(x, gate_w, W1, b1, W2, b2):
    global LAST_RESULTS
    xt2 = np.ascontiguousarray(x.reshape(-1, D_MODEL)).astype(np.float32)
    n = xt2.shape[0]

    # --- host router (tiny: [N,1024]@[1024,8]) ---
    logits = xt2 @ gate_w.astype(np.float32).T
    logits -= logits.max(axis=-1, keepdims=True)
    probs = np.exp(logits)
    probs /= probs.sum(axis=-1, keepdims=True)
    top2 = np.argsort(-probs, axis=-1, kind="stable")[:, :TOP_K]
    wt = np.take_along_axis(probs, top2, axis=-1)
    wt = wt / (wt.sum(axis=-1, keepdims=True) + 1e-9)

    # --- dispatch: sort (token, expert) pairs by expert ---
    flat_e = top2.ravel()
    flat_t = np.repeat(np.arange(n), TOP_K)
    flat_w = wt.ravel()
    order = np.argsort(flat_e, kind="stable")
    e_sorted = flat_e[order]
    t_sorted = flat_t[order]
    w_sorted = flat_w[order]
    counts = np.bincount(e_sorted, minlength=N_EXPERTS)
    starts = np.zeros(N_EXPERTS + 1, dtype=np.int64)
    starts[1:] = np.cumsum(counts)

    c = max(CAPACITY, int(-(-counts.max() // 128)) * 128)
    # slot of each sorted pair in the concatenated [E*C] dispatch buffer,
    # then invert to per-token pair slots for the final combine
    slot = np.arange(TOP_K * n) - starts[e_sorted] + e_sorted * c
    pair_slot = np.empty(TOP_K * n, dtype=np.int64)
    pair_slot[order] = slot
    pair_slot = pair_slot.reshape(n, TOP_K)

    w1f = np.asarray(W1, dtype=np.float32)
    w2f = np.asarray(W2, dtype=np.float32)
    b1f = np.asarray(b1, dtype=np.float32)
    b2f = np.asarray(b2, dtype=np.float32)

    in_maps = []
    for e in range(N_EXPERTS):
        sel = t_sorted[starts[e]:starts[e + 1]]
        ne = len(sel)
        xe = np.zeros((D_MODEL, c), dtype=_BF)
        xe[:, :ne] = xt2[sel].T.astype(_BF)
        se = np.zeros(c, dtype=np.float32)
        se[:ne] = w_sorted[starts[e]:starts[e + 1]]
        se = np.ascontiguousarray(se.reshape(c // 128, 128).T)
        in_maps.append({
            "xt": xe,
            "w1t": np.ascontiguousarray(w1f[e].T).astype(_BF),
            "w2t": np.ascontiguousarray(w2f[e].T).astype(_BF),
            "b1": np.ascontiguousarray(b1f[e].reshape(D_FF // 128, 128).T),
            "b2bc": np.ascontiguousarray(
                np.broadcast_to(b2f[e], (128, D_MODEL))),
            "s": se,
        })

    nc = _get_graph(c)
    res = None
    for attempt in range(4):
        try:
            res = run_bass_kernel_spmd(nc, in_maps,
                                       core_ids=list(range(N_CORES)),
                                       trace=TRACE and attempt < 3)
            break
        except Exception:
            # Transient device failures (NRT_EXEC_UNIT_UNRECOVERABLE, axon
            # profile-start) clear after the terminal resets; back off and
            # retry, dropping the profiling request on the last attempt.
            if attempt == 3:
                raise
            time.sleep(20 * (attempt + 1))
    LAST_RESULTS = res

    y_all = np.concatenate([res.results[e]["out"] for e in range(N_EXPERTS)],
                           axis=0)
    combined = y_all[pair_slot[:, 0]] + y_all[pair_slot[:, 1]]
    return combined.reshape(x.shape).astype(np.float32)


# revision 3
# speedup vs baseline: 1.0043x; 1.0043x over previous
"""MoE AdaptiveExpertLayer on 8 TRN2 NeuronCores (expert-parallel Bass kernel).

Sharding (hardcoded): expert-parallel — core e owns expert e's W1/b1/W2/b2.
The router (gate matmul + softmax + top-2, ~0.01% of total FLOPs) runs on the
host during input sharding; tokens are dispatched to their two chosen experts'
cores as capacity-padded batches ("all-to-all dispatch by router choice" done
at the sharding step).  Each core runs the expert MLP
    y = (relu(x @ W1.T + b1) @ W2.T + b2) * combine_weight
over its C dispatched tokens, in bf16 with fp32 PSUM accumulation, weights
fully SBUF-resident.  The host sums each token's two expert contributions.

Problem shapes: x [4, 2048, 1024], W1 [8, 4096, 1024], W2 [8, 1024, 4096].

Performance notes (from NTFF traces):
- The PE matmul stream (2368 matmuls, free dim 512) runs back-to-back at
  ~216ns each ONLY when SBUF DMA traffic is coalesced; fragmented descriptors
  slow the stream to ~259ns/matmul.  All bulk tensors are therefore staged on
  the host so every DMA moves a contiguous-per-partition slab (>=2KB
  descriptors), one instruction per x block / w1 segment / w2 quarter.
- w1 loads in column segments (first segment narrow) so the first psum tile
  only waits for ~0.5MB before the PE starts.
- pp1 has 6 PSUM bufs: with 5, the first matmul of each m1 tile periodically
  lost one matmul slot waiting on the relu (scalar engine) to free a buffer.
"""

import time

import numpy as np
import ml_dtypes
from contextlib import ExitStack

import concourse.tile as tile
from concourse import bacc, mybir
from concourse.tile import add_dep_helper
from concourse.bass_utils import run_bass_kernel_spmd

D_MODEL = 1024
D_FF = 4096
N_EXPERTS = 8
TOP_K = 2
N_CORES = 8
CAPACITY = 2176  # default per-expert token capacity (multiple of 128)

BF16 = mybir.dt.bfloat16
F32 = mybir.dt.float32
_BF = ml_dtypes.bfloat16

# Set by callers that want NTFF profiling; BASS_TRACE=1 env also works.
TRACE = False
LAST_RESULTS = None

_graph_cache = {}

N_K1 = D_MODEL // 128   # 8  contraction chunks for matmul 1
N_M1 = D_FF // 128      # 32 output tiles for matmul 1
N_DN = D_MODEL // 512   # 2  output column tiles for matmul 2

# w1 column segments: narrow first so the PE can start early
SEG_BOUNDS = [0, 128, 512, 1024, 1536, 2048, 2560, 3072, 3584, D_FF]


def _token_blocks(c):
    """Split capacity into matmul token-blocks of <=512 (multiples of 128)."""
    blocks = []
    t0 = 0
    while t0 < c:
        tb = min(512, c - t0)
        blocks.append((t0, tb))
        t0 += tb
    return blocks


def _w1_col(lo, hi, m, k):
    """SBUF column of lhsT tile (m,k) inside the segment [lo,hi) slab."""
    return N_K1 * lo + k * (hi - lo) + (m * 128 - lo)


def _build_graph(c):
    """Build + compile the per-core expert-MLP Bass graph for capacity c."""
    nc = bacc.Bacc("TRN2", target_bir_lowering=False, debug=False,
                   num_devices=N_CORES)

    # All bulk inputs are host-staged [128, ...] slabs whose DRAM layout
    # matches the SBUF destination exactly -> contiguous descriptors.
    xs = nc.dram_tensor("xs", [128, N_K1 * c], BF16, kind="ExternalInput").ap()
    w1s = nc.dram_tensor("w1s", [128, N_K1 * D_FF], BF16,
                         kind="ExternalInput").ap()
    w2s = nc.dram_tensor("w2s", [128, N_M1 * D_MODEL], BF16,
                         kind="ExternalInput").ap()
    b1 = nc.dram_tensor("b1", [128, D_FF // 128], F32, kind="ExternalInput").ap()
    b2bc = nc.dram_tensor("b2bc", [128, D_MODEL], F32, kind="ExternalInput").ap()
    s = nc.dram_tensor("s", [128, c // 128], F32, kind="ExternalInput").ap()
    out = nc.dram_tensor("out", [c, D_MODEL], F32, kind="ExternalOutput").ap()

    with tile.TileContext(nc) as tc, ExitStack() as ctx:
        wp1 = ctx.enter_context(tc.tile_pool(name="w1", bufs=1))
        wp2 = ctx.enter_context(tc.tile_pool(name="w2", bufs=1))
        cpool = ctx.enter_context(tc.tile_pool(name="consts", bufs=2))
        b2pool = ctx.enter_context(tc.tile_pool(name="b2p", bufs=1))
        xpool = ctx.enter_context(tc.tile_pool(name="x", bufs=2))
        hpool = ctx.enter_context(tc.tile_pool(name="h", bufs=N_M1))
        opool = ctx.enter_context(tc.tile_pool(name="o", bufs=4))
        pp1 = ctx.enter_context(tc.tile_pool(name="p1", bufs=6, space="PSUM"))
        pp2 = ctx.enter_context(tc.tile_pool(name="p2", bufs=2, space="PSUM"))

        blocks = _token_blocks(c)

        # w1: one [128, 8*4096] tile, loaded in contiguous column-segment
        # slabs (seg s occupies SBUF/DRAM cols [8*lo, 8*hi)).
        w1b = wp1.tile([128, N_K1 * D_FF], BF16, tag="w1", name="w1b")
        for lo, hi in zip(SEG_BOUNDS[:-1], SEG_BOUNDS[1:]):
            nc.sync.dma_start(w1b[:, N_K1 * lo:N_K1 * hi],
                              w1s[:, N_K1 * lo:N_K1 * hi])

        b1_all = cpool.tile([128, N_M1], F32, tag="b1a", name="b1a")
        nc.gpsimd.dma_start(b1_all[:], b1[:, :])
        b1_tiles = [b1_all[:, m:m + 1] for m in range(N_M1)]

        # x block 0 on the scalar queue so it lands in parallel with w1 seg 0
        t0_0, tb_0 = blocks[0]
        x0 = xpool.tile([128, N_K1 * tb_0], BF16, tag="x", name="x0")
        nc.scalar.dma_start(x0[:], xs[:, N_K1 * t0_0:N_K1 * (t0_0 + tb_0)])

        w2b = wp2.tile([128, N_M1 * D_MODEL], BF16, tag="w2", name="w2b")
        s_all = cpool.tile([128, c // 128], F32, tag="sa", name="sa")
        b2_tile = b2pool.tile([128, D_MODEL], F32, tag="b2", name="b2t")

        x_tiles = x0
        x_next = None
        first = True
        for bi, (t0, tb) in enumerate(blocks):
            if not first:
                x_tiles = x_next

            # prefetch next block's x while this block computes
            if bi + 1 < len(blocks):
                tn0, tnb = blocks[bi + 1]
                x_next = xpool.tile([128, N_K1 * tnb], BF16, tag="x",
                                    name=f"x{bi + 1}")
                nc.sync.dma_start(x_next[:],
                                  xs[:, N_K1 * tn0:N_K1 * (tn0 + tnb)])

            # h.T [D_FF, tb] = relu(W1 @ x.T + b1), FF on partitions
            h_tiles = []
            relu_insts = []
            for m in range(N_M1):
                lo, hi = next((lo, hi) for lo, hi in
                              zip(SEG_BOUNDS[:-1], SEG_BOUNDS[1:])
                              if lo <= m * 128 < hi)
                ps = pp1.tile([128, tb], F32, tag="p1", name=f"p1_{bi}_{m}")
                for k in range(N_K1):
                    col = _w1_col(lo, hi, m, k)
                    nc.tensor.matmul(
                        ps[:],
                        lhsT=w1b[:, col:col + 128],
                        rhs=x_tiles[:, k * tb:(k + 1) * tb],
                        start=(k == 0), stop=(k == N_K1 - 1))
                h = hpool.tile([128, tb], BF16, tag="h", name=f"h_{bi}_{m}")
                ri = nc.scalar.activation(h[:], ps[:],
                                          mybir.ActivationFunctionType.Relu,
                                          bias=b1_tiles[m][:])
                relu_insts.append(ri)
                h_tiles.append(h)

            if first:
                # w2 / s / b2 only gate matmul 2 — load them behind early m1
                # on the gpsimd queue so the w1 segment loads (which the PE
                # is waiting on) get the HBM bandwidth first.
                first = False
                qw = N_M1 * D_MODEL // 4
                for kc in range(4):
                    d = nc.gpsimd.dma_start(w2b[:, kc * qw:(kc + 1) * qw],
                                            w2s[:, kc * qw:(kc + 1) * qw])
                    add_dep_helper(d.ins, relu_insts[4].ins, sync=True,
                                   reason="w2 load behind early m1")
                nc.gpsimd.dma_start(s_all[:], s[:, :])
                nc.gpsimd.dma_start(b2_tile[:], b2bc[:, :])

            # y [tb, D_MODEL] = (h @ W2.T + b2) * s, tokens on partitions
            for tm in range(tb // 128):
                g = (t0 + tm * 128) // 128
                for dn in range(N_DN):
                    ps = pp2.tile([128, 512], F32, tag="p2",
                                  name=f"p2_{bi}_{tm}_{dn}")
                    for k in range(N_M1):
                        nc.tensor.matmul(
                            ps[:], lhsT=h_tiles[k][:, tm * 128:(tm + 1) * 128],
                            rhs=w2b[:, k * D_MODEL + dn * 512:
                                    k * D_MODEL + (dn + 1) * 512],
                            start=(k == 0), stop=(k == N_M1 - 1))
                    t = opool.tile([128, 512], F32, tag="t",
                                   name=f"t_{bi}_{tm}_{dn}")
                    nc.vector.tensor_add(t[:], ps[:],
                                         b2_tile[:, dn * 512:(dn + 1) * 512])
                    o = opool.tile([128, 512], F32, tag="o",
                                   name=f"o_{bi}_{tm}_{dn}")
                    nc.scalar.mul(o[:], t[:], s_all[:, g:g + 1])
                    nc.sync.dma_start(
                        out[t0 + tm * 128:t0 + (tm + 1) * 128,
                            dn * 512:(dn + 1) * 512],
                        o[:])

    nc.compile()
    return nc


def _get_graph(c):
    if c not in _graph_cache:
        _graph_cache[c] = _build_graph(c)
    return _graph_cache[c]


def _stage_w1(w1e):
    """[4096, 1024] W1 -> [128, 8*4096] segment-contiguous slab."""
    # arr[p, k, f] = W1.T[k*128+p, f] = W1[f, k*128+p]
    arr = w1e.T.reshape(N_K1, 128, D_FF).transpose(1, 0, 2)  # [128, 8, 4096]
    segs = [arr[:, :, lo:hi].reshape(128, -1)
            for lo, hi in zip(SEG_BOUNDS[:-1], SEG_BOUNDS[1:])]
    return np.ascontiguousarray(np.concatenate(segs, axis=1))


def _stage_w2(w2e):
    """[1024, 4096] W2 -> [128, 32*1024] k-contiguous slab."""
    # arr[p, k, f] = W2.T[k*128+p, f] = W2[f, k*128+p]
    arr = w2e.T.reshape(N_M1, 128, D_MODEL).transpose(1, 0, 2)
    return np.ascontiguousarray(arr.reshape(128, -1))


def _stage_x(xe_t, blocks):
    """[1024, c] x.T -> [128, 8*c] block-contiguous slab."""
    arr = xe_t.reshape(N_K1, 128, -1)  # [k, p, t]
    slabs = [np.ascontiguousarray(arr[:, :, t0:t0 + tb].transpose(1, 0, 2)
                                  ).reshape(128, -1)
             for t0, tb in blocks]
    return np.ascontiguousarray(np.concatenate(slabs, axis=1))


def kernel(x, gate_w, W1, b1, W2, b2):
    global LAST_RESULTS
    xt2 = np.ascontiguousarray(x.reshape(-1, D_MODEL)).astype(np.float32)
    n = xt2.shape[0]

    # --- host router (tiny: [N,1024]@[1024,8]) ---
    logits = xt2 @ gate_w.astype(np.float32).T
    logits -= logits.max(axis=-1, keepdims=True)
    probs = np.exp(logits)
    probs /= probs.sum(axis=-1, keepdims=True)
    top2 = np.argsort(-probs, axis=-1, kind="stable")[:, :TOP_K]
    wt = np.take_along_axis(probs, top2, axis=-1)
    wt = wt / (wt.sum(axis=-1, keepdims=True) + 1e-9)

    # --- dispatch: sort (token, expert) pairs by expert ---
    flat_e = top2.ravel()
    flat_t = np.repeat(np.arange(n), TOP_K)
    flat_w = wt.ravel()
    order = np.argsort(flat_e, kind="stable")
    e_sorted = flat_e[order]
    t_sorted = flat_t[order]
    w_sorted = flat_w[order]
    counts = np.bincount(e_sorted, minlength=N_EXPERTS)
    starts = np.zeros(N_EXPERTS + 1, dtype=np.int64)
    starts[1:] = np.cumsum(counts)

    c = max(CAPACITY, int(-(-counts.max() // 128)) * 128)
    # slot of each sorted pair in the concatenated [E*C] dispatch buffer,
    # then invert to per-token pair slots for the final combine
    slot = np.arange(TOP_K * n) - starts[e_sorted] + e_sorted * c
    pair_slot = np.empty(TOP_K * n, dtype=np.int64)
    pair_slot[order] = slot
    pair_slot = pair_slot.reshape(n, TOP_K)

    blocks = _token_blocks(c)
    w1f = np.asarray(W1, dtype=np.float32)
    w2f = np.asarray(W2, dtype=np.float32)
    b1f = np.asarray(b1, dtype=np.float32)
    b2f = np.asarray(b2, dtype=np.float32)

    in_maps = []
    for e in range(N_EXPERTS):
        sel = t_sorted[starts[e]:starts[e + 1]]
        ne = len(sel)
        xe = np.zeros((D_MODEL, c), dtype=_BF)
        xe[:, :ne] = xt2[sel].T.astype(_BF)
        se = np.zeros(c, dtype=np.float32)
        se[:ne] = w_sorted[starts[e]:starts[e + 1]]
        se = np.ascontiguousarray(se.reshape(c // 128, 128).T)
        in_maps.append({
            "xs": _stage_x(xe, blocks),
            "w1s": _stage_w1(w1f[e].astype(_BF)),
            "w2s": _stage_w2(w2f[e].astype(_BF)),
            "b1": np.ascontiguousarray(b1f[e].reshape(D_FF // 128, 128).T),
            "b2bc": np.ascontiguousarray(
                np.broadcast_to(b2f[e], (128, D_MODEL))),
            "s": se,
        })

    nc = _get_graph(c)
    res = None
    for attempt in range(4):
        try:
            res = run_bass_kernel_spmd(nc, in_maps,
                                       core_ids=list(range(N_CORES)),
                                       trace=TRACE and attempt < 3)
            break
        except Exception:
            # Transient device failures (NRT_EXEC_UNIT_UNRECOVERABLE, axon
            # profile-start) clear after the terminal resets; back off and
            # retry, dropping the profiling request on the last attempt.
            if attempt == 3:
                raise
            time.sleep(20 * (attempt + 1))
    LAST_RESULTS = res

    y_all = np.concatenate([res.results[e]["out"] for e in range(N_EXPERTS)],
                           axis=0)
    combined = y_all[pair_slot[:, 0]] + y_all[pair_slot[:, 1]]
    return combined.reshape(x.shape).astype(np.float32)


# revision 8
# speedup vs baseline: 1.0059x; 1.0016x over previous
"""MoE AdaptiveExpertLayer on 8 TRN2 NeuronCores (expert-parallel Bass kernel).

Sharding (hardcoded): expert-parallel — core e owns expert e's W1/b1/W2/b2.
The router (gate matmul + softmax + top-2, ~0.01% of total FLOPs) runs on the
host during input sharding; tokens are dispatched to their two chosen experts'
cores as capacity-padded batches ("all-to-all dispatch by router choice" done
at the sharding step).  Each core runs the expert MLP
    y = (relu(x @ W1.T + b1) @ W2.T + b2) * combine_weight
over its C dispatched tokens, in bf16 with fp32 PSUM accumulation, weights
fully SBUF-resident.  The host sums each token's two expert contributions.

Problem shapes: x [4, 2048, 1024], W1 [8, 4096, 1024], W2 [8, 1024, 4096].

Performance notes (from NTFF traces):
- The PE matmul stream (2368 matmuls, free dim 512) runs back-to-back at
  ~216ns each ONLY when SBUF DMA traffic is coalesced; fragmented descriptors
  slow the stream to ~259ns/matmul.  All bulk tensors are therefore staged on
  the host so every DMA moves a contiguous-per-partition slab (>=2KB
  descriptors), one instruction per x block / w1 segment / w2 quarter.
- w1 loads in column segments (first segment narrow) so the first psum tile
  only waits for ~0.5MB before the PE starts.
- pp1 has 6 PSUM bufs: with 5, the first matmul of each m1 tile periodically
  lost one matmul slot waiting on the relu (scalar engine) to free a buffer.
"""

import time

import numpy as np
import ml_dtypes
from contextlib import ExitStack

import concourse.tile as tile
from concourse import bacc, mybir
from concourse.tile import add_dep_helper
from concourse.bass_utils import run_bass_kernel_spmd

D_MODEL = 1024
D_FF = 4096
N_EXPERTS = 8
TOP_K = 2
N_CORES = 8
CAPACITY = 2176  # default per-expert token capacity (multiple of 128)

BF16 = mybir.dt.bfloat16
F32 = mybir.dt.float32
_BF = ml_dtypes.bfloat16

# Set by callers that want NTFF profiling; BASS_TRACE=1 env also works.
TRACE = False
LAST_RESULTS = None

_graph_cache = {}

N_K1 = D_MODEL // 128   # 8  contraction chunks for matmul 1
N_M1 = D_FF // 128      # 32 output tiles for matmul 1
N_DN = D_MODEL // 512   # 2  output column tiles for matmul 2

# w1 column segments: narrow first so the PE can start early
SEG_BOUNDS = [0, 128, 512, 1024, 1536, 2048, 2560, 3072, 3584, D_FF]


def _token_blocks(c):
    """Split capacity into matmul token-blocks of <=512 (multiples of 128)."""
    blocks = []
    t0 = 0
    while t0 < c:
        tb = min(512, c - t0)
        blocks.append((t0, tb))
        t0 += tb
    return blocks


def _w1_col(lo, hi, m, k):
    """SBUF column of lhsT tile (m,k) inside the segment [lo,hi) slab."""
    return N_K1 * lo + k * (hi - lo) + (m * 128 - lo)


def _build_graph(c):
    """Build + compile the per-core expert-MLP Bass graph for capacity c."""
    nc = bacc.Bacc("TRN2", target_bir_lowering=False, debug=False,
                   num_devices=N_CORES)

    # All bulk inputs are host-staged [128, ...] slabs whose DRAM layout
    # matches the SBUF destination exactly -> contiguous descriptors.
    xs = nc.dram_tensor("xs", [128, N_K1 * c], BF16, kind="ExternalInput").ap()
    w1s = nc.dram_tensor("w1s", [128, N_K1 * D_FF], BF16,
                         kind="ExternalInput").ap()
    w2s = nc.dram_tensor("w2s", [128, N_M1 * D_MODEL], BF16,
                         kind="ExternalInput").ap()
    b1 = nc.dram_tensor("b1", [128, D_FF // 128], F32, kind="ExternalInput").ap()
    b2bc = nc.dram_tensor("b2bc", [128, D_MODEL], F32, kind="ExternalInput").ap()
    s = nc.dram_tensor("s", [128, c // 128], F32, kind="ExternalInput").ap()
    out = nc.dram_tensor("out", [c, D_MODEL], BF16, kind="ExternalOutput").ap()

    with tile.TileContext(nc) as tc, ExitStack() as ctx:
        wp1 = ctx.enter_context(tc.tile_pool(name="w1", bufs=1))
        wp2 = ctx.enter_context(tc.tile_pool(name="w2", bufs=1))
        cpool = ctx.enter_context(tc.tile_pool(name="consts", bufs=2))
        b2pool = ctx.enter_context(tc.tile_pool(name="b2p", bufs=1))
        xpool = ctx.enter_context(tc.tile_pool(name="x", bufs=2))
        hpool = ctx.enter_context(tc.tile_pool(name="h", bufs=N_M1))
        opool = ctx.enter_context(tc.tile_pool(name="o", bufs=4))
        pp1 = ctx.enter_context(tc.tile_pool(name="p1", bufs=6, space="PSUM"))
        pp2 = ctx.enter_context(tc.tile_pool(name="p2", bufs=2, space="PSUM"))

        blocks = _token_blocks(c)

        # x block 0 first in the sync HWDGE ring: its packets drain ahead of
        # the w1 stream, so the PE's two first-matmul inputs (x0, w1 seg 0)
        # complete back to back instead of x0 starving behind 8MB of w1.
        t0_0, tb_0 = blocks[0]
        x0 = xpool.tile([128, N_K1 * tb_0], BF16, tag="x", name="x0")
        nc.sync.dma_start(x0[:], xs[:, N_K1 * t0_0:N_K1 * (t0_0 + tb_0)])

        # w1: one [128, 8*4096] tile, loaded in contiguous column-segment
        # slabs (seg s occupies SBUF/DRAM cols [8*lo, 8*hi)).
        w1b = wp1.tile([128, N_K1 * D_FF], BF16, tag="w1", name="w1b")
        for lo, hi in zip(SEG_BOUNDS[:-1], SEG_BOUNDS[1:]):
            nc.sync.dma_start(w1b[:, N_K1 * lo:N_K1 * hi],
                              w1s[:, N_K1 * lo:N_K1 * hi])

        b1_all = cpool.tile([128, N_M1], F32, tag="b1a", name="b1a")
        nc.gpsimd.dma_start(b1_all[:], b1[:, :])
        b1_tiles = [b1_all[:, m:m + 1] for m in range(N_M1)]

        # PE warm-up: the HAM clock gate holds the PE at 1.2GHz until it has
        # seen ~3.4us of sustained activity.  Run throwaway matmuls on
        # memset scratch while the first DMAs land so the real stream starts
        # at full clock.
        wsc = cpool.tile([128, 640], BF16, tag="wsc", name="wsc")
        nc.gpsimd.memset(wsc[:], 0)
        ps_w = pp2.tile([128, 512], F32, tag="p2", name="ps_warm")
        for wi in range(10):
            nc.tensor.matmul(ps_w[:], lhsT=wsc[:, 0:128], rhs=wsc[:, 128:640],
                             start=(wi == 0), stop=(wi == 9))
        wsk = cpool.tile([128, 512], F32, tag="wsk", name="wsk")
        nc.vector.tensor_copy(wsk[:], ps_w[:])

        w2b = wp2.tile([128, N_M1 * D_MODEL], BF16, tag="w2", name="w2b")
        s_all = cpool.tile([128, c // 128], F32, tag="sa", name="sa")
        b2_tile = b2pool.tile([128, D_MODEL], F32, tag="b2", name="b2t")

        x_tiles = x0
        x_next = None
        first = True
        for bi, (t0, tb) in enumerate(blocks):
            if not first:
                x_tiles = x_next

            # prefetch next block's x while this block computes
            if bi + 1 < len(blocks):
                tn0, tnb = blocks[bi + 1]
                x_next = xpool.tile([128, N_K1 * tnb], BF16, tag="x",
                                    name=f"x{bi + 1}")
                nc.sync.dma_start(x_next[:],
                                  xs[:, N_K1 * tn0:N_K1 * (tn0 + tnb)])

            # h.T [D_FF, tb] = relu(W1 @ x.T + b1), FF on partitions
            h_tiles = []
            relu_insts = []
            for m in range(N_M1):
                lo, hi = next((lo, hi) for lo, hi in
                              zip(SEG_BOUNDS[:-1], SEG_BOUNDS[1:])
                              if lo <= m * 128 < hi)
                ps = pp1.tile([128, tb], F32, tag="p1", name=f"p1_{bi}_{m}")
                for k in range(N_K1):
                    col = _w1_col(lo, hi, m, k)
                    nc.tensor.matmul(
                        ps[:],
                        lhsT=w1b[:, col:col + 128],
                        rhs=x_tiles[:, k * tb:(k + 1) * tb],
                        start=(k == 0), stop=(k == N_K1 - 1))
                h = hpool.tile([128, tb], BF16, tag="h", name=f"h_{bi}_{m}")
                ri = nc.scalar.activation(h[:], ps[:],
                                          mybir.ActivationFunctionType.Relu,
                                          bias=b1_tiles[m][:])
                relu_insts.append(ri)
                h_tiles.append(h)

            if first:
                # w2 / s / b2 only gate matmul 2 — load them behind early m1
                # on the gpsimd queue so the w1 segment loads (which the PE
                # is waiting on) get the HBM bandwidth first.
                first = False
                qw = N_M1 * D_MODEL // 4
                for kc in range(4):
                    d = nc.gpsimd.dma_start(w2b[:, kc * qw:(kc + 1) * qw],
                                            w2s[:, kc * qw:(kc + 1) * qw])
                    add_dep_helper(d.ins, relu_insts[4].ins, sync=True,
                                   reason="w2 load behind early m1")
                nc.gpsimd.dma_start(s_all[:], s[:, :])
                nc.gpsimd.dma_start(b2_tile[:], b2bc[:, :])

            # y [tb, D_MODEL] = (h @ W2.T + b2) * s, tokens on partitions
            for tm in range(tb // 128):
                g = (t0 + tm * 128) // 128
                for dn in range(N_DN):
                    ps = pp2.tile([128, 512], F32, tag="p2",
                                  name=f"p2_{bi}_{tm}_{dn}")
                    for k in range(N_M1):
                        nc.tensor.matmul(
                            ps[:], lhsT=h_tiles[k][:, tm * 128:(tm + 1) * 128],
                            rhs=w2b[:, k * D_MODEL + dn * 512:
                                    k * D_MODEL + (dn + 1) * 512],
                            start=(k == 0), stop=(k == N_M1 - 1))
                    t = opool.tile([128, 512], F32, tag="t",
                                   name=f"t_{bi}_{tm}_{dn}")
                    nc.vector.tensor_add(t[:], ps[:],
                                         b2_tile[:, dn * 512:(dn + 1) * 512])
                    o = opool.tile([128, 512], BF16, tag="o",
                                   name=f"o_{bi}_{tm}_{dn}")
                    nc.scalar.mul(o[:], t[:], s_all[:, g:g + 1])
                    nc.sync.dma_start(
                        out[t0 + tm * 128:t0 + (tm + 1) * 128,
                            dn * 512:(dn + 1) * 512],
                        o[:])

    nc.compile()
    return nc


def _get_graph(c):
    if c not in _graph_cache:
        _graph_cache[c] = _build_graph(c)
    return _graph_cache[c]


def _stage_w1(w1e):
    """[4096, 1024] W1 -> [128, 8*4096] segment-contiguous slab."""
    # arr[p, k, f] = W1.T[k*128+p, f] = W1[f, k*128+p]
    arr = w1e.T.reshape(N_K1, 128, D_FF).transpose(1, 0, 2)  # [128, 8, 4096]
    segs = [arr[:, :, lo:hi].reshape(128, -1)
            for lo, hi in zip(SEG_BOUNDS[:-1], SEG_BOUNDS[1:])]
    return np.ascontiguousarray(np.concatenate(segs, axis=1))


def _stage_w2(w2e):
    """[1024, 4096] W2 -> [128, 32*1024] k-contiguous slab."""
    # arr[p, k, f] = W2.T[k*128+p, f] = W2[f, k*128+p]
    arr = w2e.T.reshape(N_M1, 128, D_MODEL).transpose(1, 0, 2)
    return np.ascontiguousarray(arr.reshape(128, -1))


def _stage_x(xe_t, blocks):
    """[1024, c] x.T -> [128, 8*c] block-contiguous slab."""
    arr = xe_t.reshape(N_K1, 128, -1)  # [k, p, t]
    slabs = [np.ascontiguousarray(arr[:, :, t0:t0 + tb].transpose(1, 0, 2)
                                  ).reshape(128, -1)
             for t0, tb in blocks]
    return np.ascontiguousarray(np.concatenate(slabs, axis=1))


def kernel(x, gate_w, W1, b1, W2, b2):
    global LAST_RESULTS
    xt2 = np.ascontiguousarray(x.reshape(-1, D_MODEL)).astype(np.float32)
    n = xt2.shape[0]

    # --- host router (tiny: [N,1024]@[1024,8]) ---
    logits = xt2 @ gate_w.astype(np.float32).T
    logits -= logits.max(axis=-1, keepdims=True)
    probs = np.exp(logits)
    probs /= probs.sum(axis=-1, keepdims=True)
    top2 = np.argsort(-probs, axis=-1, kind="stable")[:, :TOP_K]
    wt = np.take_along_axis(probs, top2, axis=-1)
    wt = wt / (wt.sum(axis=-1, keepdims=True) + 1e-9)

    # --- dispatch: sort (token, expert) pairs by expert ---
    flat_e = top2.ravel()
    flat_t = np.repeat(np.arange(n), TOP_K)
    flat_w = wt.ravel()
    order = np.argsort(flat_e, kind="stable")
    e_sorted = flat_e[order]
    t_sorted = flat_t[order]
    w_sorted = flat_w[order]
    counts = np.bincount(e_sorted, minlength=N_EXPERTS)
    starts = np.zeros(N_EXPERTS + 1, dtype=np.int64)
    starts[1:] = np.cumsum(counts)

    c = max(CAPACITY, int(-(-counts.max() // 128)) * 128)
    # slot of each sorted pair in the concatenated [E*C] dispatch buffer,
    # then invert to per-token pair slots for the final combine
    slot = np.arange(TOP_K * n) - starts[e_sorted] + e_sorted * c
    pair_slot = np.empty(TOP_K * n, dtype=np.int64)
    pair_slot[order] = slot
    pair_slot = pair_slot.reshape(n, TOP_K)

    blocks = _token_blocks(c)
    w1f = np.asarray(W1, dtype=np.float32)
    w2f = np.asarray(W2, dtype=np.float32)
    b1f = np.asarray(b1, dtype=np.float32)
    b2f = np.asarray(b2, dtype=np.float32)

    in_maps = []
    for e in range(N_EXPERTS):
        sel = t_sorted[starts[e]:starts[e + 1]]
        ne = len(sel)
        xe = np.zeros((D_MODEL, c), dtype=_BF)
        xe[:, :ne] = xt2[sel].T.astype(_BF)
        se = np.zeros(c, dtype=np.float32)
        se[:ne] = w_sorted[starts[e]:starts[e + 1]]
        se = np.ascontiguousarray(se.reshape(c // 128, 128).T)
        in_maps.append({
            "xs": _stage_x(xe, blocks),
            "w1s": _stage_w1(w1f[e].astype(_BF)),
            "w2s": _stage_w2(w2f[e].astype(_BF)),
            "b1": np.ascontiguousarray(b1f[e].reshape(D_FF // 128, 128).T),
            "b2bc": np.ascontiguousarray(
                np.broadcast_to(b2f[e], (128, D_MODEL))),
            "s": se,
        })

    nc = _get_graph(c)
    res = None
    for attempt in range(4):
        try:
            res = run_bass_kernel_spmd(nc, in_maps,
                                       core_ids=list(range(N_CORES)),
                                       trace=TRACE and attempt < 3)
            break
        except Exception:
            # Transient device failures (NRT_EXEC_UNIT_UNRECOVERABLE, axon
            # profile-start) clear after the terminal resets; back off and
            # retry, dropping the profiling request on the last attempt.
            if attempt == 3:
                raise
            time.sleep(20 * (attempt + 1))
    LAST_RESULTS = res

    y_all = np.concatenate([res.results[e]["out"] for e in range(N_EXPERTS)],
                           axis=0)
    combined = (y_all[pair_slot[:, 0]].astype(np.float32)
                + y_all[pair_slot[:, 1]].astype(np.float32))
    return combined.reshape(x.shape).astype(np.float32)
